# revision 1
# baseline (speedup 1.0000x reference)
"""Trainium2 Bass kernel for nn_AFFWithCustomGCN (SA-GC block + BN + residual relu).

Math (per batch n):
    Ah[h]   = A[h] * attn[n,h]                         # [H,V,V]
    feat    = einsum('ctv,hvw->hctw', x[n], Ah)        # aggregate over v
    pre     = einsum('hctw,hoc->otw', feat, Wd) + bd.sum(0)
    out     = relu(bn(pre) + x[n])                     # relu(relu(y)) == relu(y)

Data-parallel over batch N=256 across 8 cores (32/core), processed in quads
(4 batches) to fill 128 partitions.

Per quad on device:
  MM1 (per 128-col chunk j of (t,vp), per batch i):
      zt[(t4 vp)=128, (h o | res)=128] = x_bf[c, chunk].T @ wdx[c, 128]
    x is the *stationary* operand (bf16 -> FWL), 4 batches ride the 4 PE
    row-groups concurrently (tile_position=(32i,0)).  wdx = [WdT | diag(1/s)]
    so column block 96:128 carries x/scale for the residual.
  MM2 (per chunk, batch, head h=0..3):
      po[(i o), (t4 w)] += zt[:, 128i+32h:+32].T @ BD_h
    BD_h is a host-built block-diagonal Ah (heads 0-2, per batch) or the
    constant identity block (h=3, residual).  Heads+residual accumulate in
    PSUM; 4 batches ride the 4 PE col-groups (tile_position=(0,32i)).
  Epilogue: one ACT op per PSUM bank: out = relu(po*scale + shift)
    (scale/shift fold BN gamma/beta/mean/var and summed conv bias).

Every PE instruction is arranged to need <=1 semaphore wait (walrus S3_LW
limit): zt copies alternate DVE/ACT by chunk parity, PSUM-slot releasers and
operand producers are kept on the same engine per class.
"""

import numpy as np

import concourse.bass as bass
import concourse.tile as tile
from concourse import mybir
from concourse import bass2jax as _b2j
from concourse.bass_utils import run_bass_kernel_spmd


def _split_multi_waits(bir_json: bytes) -> bytes:
    """Walrus allows only one sync-wait per TPB instruction on several
    queue structs.  Split any instruction with >1 wait into preceding
    single-wait EventSemaphore instructions on the same engine (pure wait
    carriers, identical semantics)."""
    import orjson
    bir = orjson.loads(bir_json)
    ctr = 0
    for fn in bir.get("functions", []):
        for blk in fn.get("blocks", []):
            insts = blk.get("instructions")
            if not insts:
                continue
            out = []
            for inst in insts:
                si = inst.get("sync_info") or {}
                waits = si.get("on_wait") or []
                if len(waits) > 1:
                    eng = inst.get("engine")
                    for w in waits[:-1]:
                        out.append({
                            "debug": inst.get("debug", 0),
                            "engine": eng, "ins": [], "outs": [],
                            "name": f"WS-{ctr}",
                            "opcode": "EventSemaphore",
                            "sync_info": {"on_update": [], "on_wait": [w]},
                        })
                        ctr += 1
                    si["on_wait"] = [waits[-1]]
                out.append(inst)
            blk["instructions"] = out
    return orjson.dumps(bir)


_orig_compile_bir = _b2j.compile_bir_kernel


def _patched_compile_bir(bir_json, tmpdir, neff_name="file.neff"):
    return _orig_compile_bir(_split_multi_waits(bir_json), tmpdir,
                             neff_name=neff_name)


if _b2j.compile_bir_kernel is not _patched_compile_bir:
    _b2j.compile_bir_kernel = _patched_compile_bir

F32 = mybir.dt.float32
BF16 = mybir.dt.bfloat16

N, C, T, V, H = 256, 32, 128, 25, 3
VP = 32                     # v padded to 32 so (t,v) chunks of 128 = 4 whole t's
TVP = T * VP                # 4096
TW = T * V                  # 3200
NCORES = 8
NSH = N // NCORES           # 32 batches per core
NQ = NSH // 4               # 8 quads per core
BN_EPS = 1e-5

_CACHE = {}


def _build_graph():
    nc = bass.Bass()

    xp_d = nc.declare_dram_parameter("xp", [NSH, C, TVP], F32, isOutput=False)
    bdall_d = nc.declare_dram_parameter("bdall", [NSH, 128, 300], BF16, isOutput=False)
    bd4_d = nc.declare_dram_parameter("bd4", [128, 100], BF16, isOutput=False)
    wdx_d = nc.declare_dram_parameter("wdx", [128, 512], BF16, isOutput=False)
    scale_d = nc.declare_dram_parameter("scale", [128, 1], F32, isOutput=False)
    shift_d = nc.declare_dram_parameter("shift", [128, 1], F32, isOutput=False)
    out_d = nc.declare_dram_parameter("out", [NSH, C, TW], F32, isOutput=True)

    with tile.TileContext(nc) as tc:
        with (
            tc.tile_pool(name="singles", bufs=1) as singles,
            tc.tile_pool(name="xpool", bufs=2) as xpool,
            tc.tile_pool(name="ztpool", bufs=4) as ztpool,
            tc.tile_pool(name="opool", bufs=2) as opool,
            tc.tile_pool(name="psPT", bufs=2, space="PSUM") as psPT,
            tc.tile_pool(name="psAcc", bufs=3, space="PSUM") as psAcc,
            tc.tile_pool(name="psJ", bufs=1, space="PSUM") as psJ,
        ):
            # constants: raw DMA loads + DVE copies so PE sees DVE-produced tiles
            wdx_raw = singles.tile([128, 512], BF16)
            nc.sync.dma_start(out=wdx_raw, in_=wdx_d[:, :])
            wdx_sb = singles.tile([128, 512], BF16)
            nc.vector.tensor_copy(wdx_sb, wdx_raw)

            bd4_raw = singles.tile([128, 100], BF16)
            nc.sync.dma_start(out=bd4_raw, in_=bd4_d[:, :])
            bd4_sb = singles.tile([128, 100], BF16)
            nc.vector.tensor_copy(bd4_sb, bd4_raw)

            scale_sb = singles.tile([128, 1], F32)
            nc.sync.dma_start(out=scale_sb, in_=scale_d[:, :])
            shift_sb = singles.tile([128, 1], F32)
            nc.sync.dma_start(out=shift_sb, in_=shift_d[:, :])

            # all per-batch block-diag Ah tensors, loaded once (no slot reuse
            # -> the load carries no waits; PE pre-touches the tile so MM2s
            # never wait on its DMA lane)
            bd_big = singles.tile([128, NSH, 300], BF16)
            nc.sync.dma_start(
                out=bd_big, in_=bdall_d.rearrange("n p f -> p n f")
            )
            junk_ps = psJ.tile([1, 4], F32)
            nc.tensor.matmul(
                junk_ps[:, 0:1], bd_big[:, 0, 0:1], bd_big[:, 0, 0:1],
                start=True, stop=True,
            )
            junk_sb = singles.tile([1, 4], BF16)

            x_hist = []
            for q in range(NQ):
                # x quad, cast f32 -> bf16 during DMA (SWDGE).  gpsimd
                # pre-touches the tile being overwritten so the DMA's WAW
                # wait is already observed and only the PE WAR wait remains.
                x_bf = xpool.tile([128, TVP], BF16)
                if len(x_hist) >= 2:
                    nc.gpsimd.tensor_copy(junk_sb, x_hist[-2][0:1, 0:4])
                x_hist.append(x_bf)
                nc.gpsimd.dma_start(
                    out=x_bf,
                    in_=xp_d[4 * q:4 * q + 4].rearrange("n c f -> (n c) f"),
                )

                out_sb = opool.tile([128, TW], F32)
                po = None
                base = 0
                for j in range(32):
                    # MM1: one K=128 matmul per chunk via block-diagonal wdx
                    # (4 batches in one instruction, single PSUM bank writer)
                    zt = ztpool.tile([128, 512], BF16)
                    pt = psPT.tile([128, 512], F32)
                    nc.tensor.matmul(
                        pt,
                        x_bf[:, 128 * j:128 * (j + 1)],
                        wdx_sb,
                        start=True, stop=True,
                    )
                    nc.vector.tensor_copy(zt[:, 0:256], pt[:, 0:256])
                    nc.scalar.activation(
                        zt[:, 256:512], pt[:, 256:512],
                        mybir.ActivationFunctionType.Copy)

                    if j % 5 == 0:
                        po = psAcc.tile([128, 512], F32)
                        base = j
                    off = (j - base) * 100
                    for h in range(3):
                        for i in range(4):
                            nc.tensor.matmul(
                                po[32 * i:32 * (i + 1), off:off + 100],
                                zt[:, 96 * i + 32 * h:96 * i + 32 * h + 32],
                                bd_big[:, 4 * q + i, 100 * h:100 * (h + 1)],
                                start=(h == 0), stop=False,
                                tile_position=(0, 32 * i),
                            )
                    # residual head: identity rhs is batch-independent ->
                    # all 4 batches in one full-width matmul (zt cols 384:512
                    # hold the residual block for all batches contiguously)
                    nc.tensor.matmul(
                        po[:, off:off + 100],
                        zt[:, 384:512],
                        bd4_sb[:, :],
                        start=False, stop=True,
                    )

                    if j % 5 == 4 or j == 31:
                        cols = off + 100
                        t0 = base * 4
                        nc.scalar.activation(
                            out_sb[:, t0 * V:t0 * V + cols], po[:, :cols],
                            mybir.ActivationFunctionType.Relu,
                            bias=shift_sb, scale=scale_sb,
                        )

                nc.sync.dma_start(
                    out=out_d[4 * q:4 * q + 4].rearrange("n c f -> (n c) f"),
                    in_=out_sb,
                )

    return nc


def _prep_host(x, A, attn, Wd, bd, bn_gamma, bn_beta, bn_mean, bn_var):
    x = np.asarray(x, dtype=np.float32)
    A = np.asarray(A, dtype=np.float32)
    attn = np.asarray(attn, dtype=np.float32)
    Wd = np.asarray(Wd, dtype=np.float32)
    bd = np.asarray(bd, dtype=np.float32)
    bn_gamma = np.asarray(bn_gamma, dtype=np.float32)
    bn_beta = np.asarray(bn_beta, dtype=np.float32)
    bn_mean = np.asarray(bn_mean, dtype=np.float32)
    bn_var = np.asarray(bn_var, dtype=np.float32)

    # pad x's v dim 25 -> 32
    xp = np.zeros((N, C, T, VP), dtype=np.float32)
    xp[:, :, :, :V] = x.reshape(N, C, T, V)
    xp = xp.reshape(N, C, TVP)

    scale = bn_gamma / np.sqrt(bn_var + BN_EPS)           # [32]
    shift = (bd.sum(axis=0) - bn_mean) * scale + bn_beta  # [32]
    ssafe = np.where(np.abs(scale) < 1e-6, 1e-6, scale)

    # block-diagonal Ah: bdall[n, 32q+v, 100h+25q'+w] = (A*attn)[n,h,v,w] δqq'
    Ah = A[None, :, :, :] * attn                  # [N,H,V,V]
    z6 = np.zeros((N, 4, VP, H, 4, V), dtype=np.float32)
    for qq in range(4):
        z6[:, qq, :V, :, qq, :] = Ah.transpose(0, 2, 1, 3)   # [n, v, h, w]
    bdall = z6.reshape(N, 128, 300).astype(ml_bf16())

    # residual identity block: bd4[32q+v, 25q'+w] = δqq' δvw / 1 (v<25)
    bd4 = np.zeros((4, VP, 4, V), dtype=np.float32)
    for qq in range(4):
        bd4[qq, :V, qq, :] = np.eye(V, dtype=np.float32)
    bd4 = bd4.reshape(128, 100).astype(ml_bf16())

    # wdx: block-diagonal [ (i c), (i, h o | res) ]; res block = diag(1/scale)
    wdt = Wd.transpose(2, 0, 1).reshape(C, H * C)         # [c, (h o)]
    wdx1 = np.concatenate(
        [wdt, np.diag((1.0 / ssafe).astype(np.float32))], axis=1)  # [32,128]
    # column order: heads for batch i at 96i..96i+96, residual blocks for
    # all batches contiguous at 384+32i (so the residual matmul's weights
    # slice is a plain 2D AP)
    wdx = np.zeros((128, 512), dtype=np.float32)
    for i in range(4):
        wdx[32 * i:32 * (i + 1), 96 * i:96 * (i + 1)] = wdx1[:, :96]
        wdx[32 * i:32 * (i + 1), 384 + 32 * i:384 + 32 * (i + 1)] = wdx1[:, 96:]
    wdx = wdx.astype(ml_bf16())                           # [128,512] bf16

    scale4 = np.tile(scale, 4).reshape(128, 1).astype(np.float32)
    shift4 = np.tile(shift, 4).reshape(128, 1).astype(np.float32)
    return xp, bdall, bd4, wdx, scale4, shift4


def ml_bf16():
    import ml_dtypes
    return ml_dtypes.bfloat16


def kernel(x, A, attn, Wd, bd, bn_gamma, bn_beta, bn_mean, bn_var,
           _trace=False):
    xp, bdall, bd4, wdx, scale4, shift4 = _prep_host(
        x, A, attn, Wd, bd, bn_gamma, bn_beta, bn_mean, bn_var)

    if "nc" not in _CACHE:
        _CACHE["nc"] = _build_graph()
    nc = _CACHE["nc"]

    in_maps = []
    for i in range(NCORES):
        sl = slice(i * NSH, (i + 1) * NSH)
        in_maps.append({
            "xp": xp[sl],
            "bdall": bdall[sl],
            "bd4": bd4,
            "wdx": wdx,
            "scale": scale4,
            "shift": shift4,
        })

    res = run_bass_kernel_spmd(
        nc, in_maps, core_ids=list(range(NCORES)), trace=_trace,
    )
    out = np.concatenate([r["out"] for r in res.results], axis=0)
    out = out.reshape(N, C, T, V)
    if _trace:
        return out, res
    return out



# revision 2
# speedup vs baseline: 1.5917x; 1.5917x over previous
"""Trainium2 Bass kernel for nn_AFFWithCustomGCN (SA-GC block + BN + residual relu).

Math (per batch n):
    Ah[h]   = A[h] * attn[n,h]                         # [H,V,V]
    feat    = einsum('ctv,hvw->hctw', x[n], Ah)        # aggregate over v
    pre     = einsum('hctw,hoc->otw', feat, Wd) + bd.sum(0)
    out     = relu(bn(pre) + x[n])                     # relu(relu(y)) == relu(y)

Data-parallel over batch N=256 across 8 cores (32/core), processed in quads
(4 batches) to fill 128 partitions.

Per quad on device (phase-split so the PE stream is dense and HAM stays warm):
  Phase 1 (32 chunks j of 128 (t,vp) cols):
      MM1: pt[(t4 vp)=128, (h-major: 4h' x 4i x 32o)=512] = x_bf[chunk].T @ wdx
      followed by one PSUM->SBUF bf16 copy per chunk (DVE/ACT by parity).
    wdx is block-diag over batches; col 128h+32i+o = Wd[h,o,c] at row 32i+c,
    residual block h'=3: col 384+32i+o = diag(1/scale).
  Phase 2 (32 chunks):
      po[(i o), (t4 w)] += zt[:, 128h+32i:+32].T @ BD_h   (h=0..2, 4 batches
      concurrently in PE col groups) + residual identity block, accumulated
      over heads in PSUM; epilogue ACT per 5 chunks:
      out = relu(po*scale + shift) with BN/bias folded into scale/shift.
"""

import numpy as np

import concourse.bass as bass
import concourse.tile as tile
from concourse import mybir
from concourse import bass2jax as _b2j
from concourse.bass_utils import run_bass_kernel_spmd


def _split_multi_waits(bir_json: bytes) -> bytes:
    """Walrus allows only one sync-wait per TPB instruction on several
    queue structs.  Split any instruction with >1 wait into preceding
    single-wait EventSemaphore instructions on the same engine (pure wait
    carriers, identical semantics)."""
    import orjson
    bir = orjson.loads(bir_json)
    ctr = 0
    for fn in bir.get("functions", []):
        for blk in fn.get("blocks", []):
            insts = blk.get("instructions")
            if not insts:
                continue
            out = []
            for inst in insts:
                si = inst.get("sync_info") or {}
                waits = si.get("on_wait") or []
                if len(waits) > 1:
                    eng = inst.get("engine")
                    for w in waits[:-1]:
                        out.append({
                            "debug": inst.get("debug", 0),
                            "engine": eng, "ins": [], "outs": [],
                            "name": f"WS-{ctr}",
                            "opcode": "EventSemaphore",
                            "sync_info": {"on_update": [], "on_wait": [w]},
                        })
                        ctr += 1
                    si["on_wait"] = [waits[-1]]
                out.append(inst)
            blk["instructions"] = out
    return orjson.dumps(bir)


_orig_compile_bir = _b2j.compile_bir_kernel


def _patched_compile_bir(bir_json, tmpdir, neff_name="file.neff"):
    return _orig_compile_bir(_split_multi_waits(bir_json), tmpdir,
                             neff_name=neff_name)


if _b2j.compile_bir_kernel is not _patched_compile_bir:
    _b2j.compile_bir_kernel = _patched_compile_bir

F32 = mybir.dt.float32
BF16 = mybir.dt.bfloat16

N, C, T, V, H = 256, 32, 128, 25, 3
VP = 32                     # v padded to 32 so (t,v) chunks of 128 = 4 whole t's
TVP = T * VP                # 4096
TW = T * V                  # 3200
NCORES = 8
NSH = N // NCORES           # 32 batches per core
NQ = NSH // 4               # 8 quads per core
BN_EPS = 1e-5

_CACHE = {}


def _build_graph():
    nc = bass.Bass()

    xp_d = nc.declare_dram_parameter("xp", [NSH, C, TVP], F32, isOutput=False)
    bdall_d = nc.declare_dram_parameter("bdall", [NSH, 128, 300], BF16, isOutput=False)
    bd4_d = nc.declare_dram_parameter("bd4", [128, 100], BF16, isOutput=False)
    wdx_d = nc.declare_dram_parameter("wdx", [128, 512], BF16, isOutput=False)
    scale_d = nc.declare_dram_parameter("scale", [128, 1], F32, isOutput=False)
    shift_d = nc.declare_dram_parameter("shift", [128, 1], F32, isOutput=False)
    out_d = nc.declare_dram_parameter("out", [NSH, C, TW], F32, isOutput=True)

    with tile.TileContext(nc) as tc:
        with (
            tc.tile_pool(name="singles", bufs=1) as singles,
            tc.tile_pool(name="xpool", bufs=2) as xpool,
            tc.tile_pool(name="ztpool", bufs=48) as ztpool,
            tc.tile_pool(name="opool", bufs=2) as opool,
            tc.tile_pool(name="psPT", bufs=4, space="PSUM") as psPT,
            tc.tile_pool(name="psAcc", bufs=3, space="PSUM") as psAcc,
        ):
            wdx_sb = singles.tile([128, 512], BF16)
            nc.sync.dma_start(out=wdx_sb, in_=wdx_d[:, :])
            bd4_sb = singles.tile([128, 100], BF16)
            nc.sync.dma_start(out=bd4_sb, in_=bd4_d[:, :])
            scale_sb = singles.tile([128, 1], F32)
            nc.sync.dma_start(out=scale_sb, in_=scale_d[:, :])
            shift_sb = singles.tile([128, 1], F32)
            nc.sync.dma_start(out=shift_sb, in_=shift_d[:, :])

            bd_big = singles.tile([128, NSH, 300], BF16)
            nc.sync.dma_start(
                out=bd_big, in_=bdall_d.rearrange("n p f -> p n f")
            )

            for q in range(NQ):
                x_bf = xpool.tile([128, TVP], BF16)
                nc.gpsimd.dma_start(
                    out=x_bf,
                    in_=xp_d[4 * q:4 * q + 4].rearrange("n c f -> (n c) f"),
                )

                # Phase 1: channel-mix matmuls for the whole quad
                zts = []
                for j in range(32):
                    pt = psPT.tile([128, 512], F32)
                    nc.tensor.matmul(
                        pt,
                        x_bf[:, 128 * j:128 * (j + 1)],
                        wdx_sb,
                        start=True, stop=True,
                    )
                    zt = ztpool.tile([128, 512], BF16)
                    if j % 2 == 0:
                        nc.vector.tensor_copy(zt, pt)
                    else:
                        nc.scalar.activation(
                            zt, pt, mybir.ActivationFunctionType.Copy)
                    zts.append(zt)

                # Phase 2: adjacency propagation + head-sum + residual
                out_sb = opool.tile([128, TW], F32)
                po = None
                base = 0
                for j in range(32):
                    zt = zts[j]
                    if j % 5 == 0:
                        po = psAcc.tile([128, 512], F32)
                        base = j
                    off = (j - base) * 100
                    for h in range(3):
                        for i in range(4):
                            nc.tensor.matmul(
                                po[32 * i:32 * (i + 1), off:off + 100],
                                zt[:, 128 * h + 32 * i:128 * h + 32 * i + 32],
                                bd_big[:, 4 * q + i, 100 * h:100 * (h + 1)],
                                start=(h == 0), stop=False,
                                tile_position=(0, 32 * i),
                            )
                    # residual: identity rhs is batch-independent -> all 4
                    # batches in one full-width matmul
                    nc.tensor.matmul(
                        po[:, off:off + 100],
                        zt[:, 384:512],
                        bd4_sb[:, :],
                        start=False, stop=True,
                    )

                    if j % 5 == 4 or j == 31:
                        cols = off + 100
                        t0 = base * 4
                        nc.scalar.activation(
                            out_sb[:, t0 * V:t0 * V + cols], po[:, :cols],
                            mybir.ActivationFunctionType.Relu,
                            bias=shift_sb, scale=scale_sb,
                        )

                nc.sync.dma_start(
                    out=out_d[4 * q:4 * q + 4].rearrange("n c f -> (n c) f"),
                    in_=out_sb,
                )

    return nc


def _prep_host(x, A, attn, Wd, bd, bn_gamma, bn_beta, bn_mean, bn_var):
    x = np.asarray(x, dtype=np.float32)
    A = np.asarray(A, dtype=np.float32)
    attn = np.asarray(attn, dtype=np.float32)
    Wd = np.asarray(Wd, dtype=np.float32)
    bd = np.asarray(bd, dtype=np.float32)
    bn_gamma = np.asarray(bn_gamma, dtype=np.float32)
    bn_beta = np.asarray(bn_beta, dtype=np.float32)
    bn_mean = np.asarray(bn_mean, dtype=np.float32)
    bn_var = np.asarray(bn_var, dtype=np.float32)

    # pad x's v dim 25 -> 32
    xp = np.zeros((N, C, T, VP), dtype=np.float32)
    xp[:, :, :, :V] = x.reshape(N, C, T, V)
    xp = xp.reshape(N, C, TVP)

    scale = bn_gamma / np.sqrt(bn_var + BN_EPS)           # [32]
    shift = (bd.sum(axis=0) - bn_mean) * scale + bn_beta  # [32]
    ssafe = np.where(np.abs(scale) < 1e-6, 1e-6, scale)

    # block-diagonal Ah: bdall[n, 32q+v, 100h+25q'+w] = (A*attn)[n,h,v,w] δqq'
    Ah = A[None, :, :, :] * attn                  # [N,H,V,V]
    z6 = np.zeros((N, 4, VP, H, 4, V), dtype=np.float32)
    for qq in range(4):
        z6[:, qq, :V, :, qq, :] = Ah.transpose(0, 2, 1, 3)   # [n, v, h, w]
    bdall = z6.reshape(N, 128, 300).astype(ml_bf16())

    # residual identity block: bd4[32q+v, 25q'+w] = δqq' δvw (v<25)
    bd4 = np.zeros((4, VP, 4, V), dtype=np.float32)
    for qq in range(4):
        bd4[qq, :V, qq, :] = np.eye(V, dtype=np.float32)
    bd4 = bd4.reshape(128, 100).astype(ml_bf16())

    # wdx: [ (i c), (h-major: 4h' x 4i x 32o) ]; h'=3 is diag(1/scale) residual
    wdt = Wd.transpose(2, 0, 1).reshape(C, H * C)         # [c, (h o)]
    rinv = np.diag((1.0 / ssafe).astype(np.float32))      # [c, o]
    wdx = np.zeros((128, 512), dtype=np.float32)
    for i in range(4):
        for h in range(H):
            wdx[32 * i:32 * (i + 1), 128 * h + 32 * i:128 * h + 32 * (i + 1)] = \
                wdt[:, 32 * h:32 * (h + 1)]
        wdx[32 * i:32 * (i + 1), 384 + 32 * i:384 + 32 * (i + 1)] = rinv
    wdx = wdx.astype(ml_bf16())                           # [128,512] bf16

    scale4 = np.tile(scale, 4).reshape(128, 1).astype(np.float32)
    shift4 = np.tile(shift, 4).reshape(128, 1).astype(np.float32)
    return xp, bdall, bd4, wdx, scale4, shift4


def ml_bf16():
    import ml_dtypes
    return ml_dtypes.bfloat16


def kernel(x, A, attn, Wd, bd, bn_gamma, bn_beta, bn_mean, bn_var,
           _trace=False):
    xp, bdall, bd4, wdx, scale4, shift4 = _prep_host(
        x, A, attn, Wd, bd, bn_gamma, bn_beta, bn_mean, bn_var)

    if "nc" not in _CACHE:
        _CACHE["nc"] = _build_graph()
    nc = _CACHE["nc"]

    in_maps = []
    for i in range(NCORES):
        sl = slice(i * NSH, (i + 1) * NSH)
        in_maps.append({
            "xp": xp[sl],
            "bdall": bdall[sl],
            "bd4": bd4,
            "wdx": wdx,
            "scale": scale4,
            "shift": shift4,
        })

    res = run_bass_kernel_spmd(
        nc, in_maps, core_ids=list(range(NCORES)), trace=_trace,
    )
    out = np.concatenate([r["out"] for r in res.results], axis=0)
    out = out.reshape(N, C, T, V)
    if _trace:
        return out, res
    return out


# revision 14
# speedup vs baseline: 1.7360x; 1.0907x over previous
"""Trainium2 Bass kernel for nn_AFFWithCustomGCN (SA-GC block + BN + residual relu).

Math (per batch n):
    Ah[h]   = A[h] * attn[n,h]                         # [H,V,V]
    feat    = einsum('ctv,hvw->hctw', x[n], Ah)        # aggregate over v
    pre     = einsum('hctw,hoc->otw', feat, Wd) + bd.sum(0)
    out     = relu(bn(pre) + x[n])                     # relu(relu(y)) == relu(y)

Data-parallel over batch N=256 across 8 cores (32/core), processed in quads
(4 batches) to fill 128 partitions.

Per quad on device (phase-split so the PE stream is dense and HAM stays warm):
  Phase 1 (32 chunks j of 128 (t,vp) cols): 4 concurrent row-group matmuls
      pt[(t4 vp)=128, 96i+32h+o] = x_bf[32i:+32, chunk].T @ wd4[32i:+32, :]
    (K=32 per batch, tile_position=(32i,0) -> 4 batches ride the PE row
    groups concurrently), then one PSUM->SBUF bf16 copy per chunk
    (DVE/ACT by parity) -> zt.
  Phase 2 (32 chunks): per head h=0..2, 4 batches in PE col groups:
      po[(i o), (t4 w)] += zt[:, 96i+32h:+32].T @ BD_h
    accumulated over heads in PSUM.
  Epilogue per 5-chunk window: DVE: tmp = po*scale + x (residual add with
  strided x read), ACT: out = relu(tmp + shift).  BN gamma/beta/mean/var and
  summed conv bias folded into scale/shift.
"""

import numpy as np

import concourse.bass as bass
import concourse.tile as tile
from concourse import mybir
from concourse import bass2jax as _b2j
from concourse.bass_utils import run_bass_kernel_spmd


def _split_multi_waits(bir_json: bytes) -> bytes:
    """Walrus allows only one sync-wait per TPB instruction on several
    queue structs.  Split any instruction with >1 wait into preceding
    single-wait EventSemaphore instructions on the same engine (pure wait
    carriers, identical semantics)."""
    import orjson
    bir = orjson.loads(bir_json)
    ctr = 0
    for fn in bir.get("functions", []):
        for blk in fn.get("blocks", []):
            insts = blk.get("instructions")
            if not insts:
                continue
            out = []
            for inst in insts:
                si = inst.get("sync_info") or {}
                waits = si.get("on_wait") or []
                if len(waits) > 1:
                    eng = inst.get("engine")
                    for w in waits[:-1]:
                        out.append({
                            "debug": inst.get("debug", 0),
                            "engine": eng, "ins": [], "outs": [],
                            "name": f"WS-{ctr}",
                            "opcode": "EventSemaphore",
                            "sync_info": {"on_update": [], "on_wait": [w]},
                        })
                        ctr += 1
                    si["on_wait"] = [waits[-1]]
                out.append(inst)
            blk["instructions"] = out
    return orjson.dumps(bir)


_orig_compile_bir = _b2j.compile_bir_kernel


def _patched_compile_bir(bir_json, tmpdir, neff_name="file.neff"):
    return _orig_compile_bir(_split_multi_waits(bir_json), tmpdir,
                             neff_name=neff_name)


if _b2j.compile_bir_kernel is not _patched_compile_bir:
    _b2j.compile_bir_kernel = _patched_compile_bir

F32 = mybir.dt.float32
BF16 = mybir.dt.bfloat16

N, C, T, V, H = 256, 32, 128, 25, 3
VP = 32                     # v padded to 32 so (t,v) chunks of 128 = 4 whole t's
TVP = T * VP                # 4096
TW = T * V                  # 3200
NCORES = 8
NSH = N // NCORES           # 32 batches per core
NQ = NSH // 4               # 8 quads per core
BN_EPS = 1e-5

import os
_BISECT_NO_STT = bool(os.environ.get("BISECT_NO_STT"))

_CACHE = {}


def _build_graph(nq=NQ):
    nc = bass.Bass()

    xp_d = nc.declare_dram_parameter("xp", [NSH, C, TVP], F32, isOutput=False)
    bdall_d = nc.declare_dram_parameter("bdall", [NSH, 128, 300], BF16, isOutput=False)
    wd4_d = nc.declare_dram_parameter("wd4", [128, 384], BF16, isOutput=False)
    scale_d = nc.declare_dram_parameter("scale", [128, 1], F32, isOutput=False)
    shift_d = nc.declare_dram_parameter("shift", [128, 1], F32, isOutput=False)
    out_d = nc.declare_dram_parameter("out", [NSH, C, TW], F32, isOutput=True)

    with tile.TileContext(nc) as tc:
        with (
            tc.tile_pool(name="singles", bufs=1) as singles,
            tc.tile_pool(name="xpool", bufs=2) as xpool,
            tc.tile_pool(name="ztpool", bufs=48) as ztpool,
            tc.tile_pool(name="opool", bufs=2) as opool,
            tc.tile_pool(name="tpool", bufs=3) as tpool,
            tc.tile_pool(name="psPT", bufs=4, space="PSUM") as psPT,
            tc.tile_pool(name="psAcc", bufs=3, space="PSUM") as psAcc,
        ):
            wd4_sb = singles.tile([128, 384], BF16)
            nc.sync.dma_start(out=wd4_sb, in_=wd4_d[:, :])
            scale_sb = singles.tile([128, 1], F32)
            nc.sync.dma_start(out=scale_sb, in_=scale_d[:, :])
            shift_sb = singles.tile([128, 1], F32)
            nc.sync.dma_start(out=shift_sb, in_=shift_d[:, :])

            bd_big = singles.tile([128, NSH, 300], BF16)
            nc.sync.dma_start(
                out=bd_big, in_=bdall_d.rearrange("n p f -> p n f")
            )

            for q in range(nq):
                x_bf = xpool.tile([128, T, VP], BF16)
                nc.gpsimd.dma_start(
                    out=x_bf,
                    in_=xp_d[4 * q:4 * q + 4].rearrange(
                        "n c (t v) -> (n c) t v", v=VP),
                )

                # Phase 1: channel-mix (block-diag wd4 zeroes cross-batch
                # terms; x chunk is the stationary operand)
                zts = []
                for j in range(32):
                    pt = psPT.tile([128, 384], F32)
                    nc.tensor.matmul(
                        pt,
                        x_bf.rearrange(
                            "p t v -> p (t v)")[:, 128 * j:128 * (j + 1)],
                        wd4_sb,
                        start=True, stop=True,
                    )
                    zt = ztpool.tile([128, 384], BF16)
                    if j % 2 == 0:
                        nc.vector.tensor_copy(zt, pt)
                    else:
                        nc.scalar.activation(
                            zt, pt, mybir.ActivationFunctionType.Copy)
                    zts.append(zt)

                # Phase 2: adjacency propagation + head-sum
                out_sb = opool.tile([128, T, V], F32)
                po = None
                base = 0
                for j in range(32):
                    zt = zts[j]
                    if j % 5 == 0:
                        po = psAcc.tile([128, 512], F32)
                        base = j
                    off = (j - base) * 100
                    # One accumulation group per (bank, col-group) WINDOW:
                    # start=True re-marks the whole 2KB bank pending-zero, so
                    # it must fire only on the window's first chunk; stop on
                    # the last.  Middle chunks are pure accumulates (pending-
                    # zero bytes overwrite), so scheduler order is free.
                    first = (j % 5 == 0)
                    last = (j % 5 == 4 or j == 31)
                    for h in range(3):
                        for i in range(4):
                            nc.tensor.matmul(
                                po[32 * i:32 * (i + 1), off:off + 100],
                                zt[:, 96 * i + 32 * h:96 * i + 32 * h + 32],
                                bd_big[:, 4 * q + i, 100 * h:100 * (h + 1)],
                                start=(h == 0 and first),
                                stop=(h == 2 and last),
                                tile_position=(0, 32 * i),
                                # CoreSim's group-check mis-addresses
                                # base-partition!=0 col-tiled outs (false
                                # conflicts); the pattern is HW-validated.
                                skip_group_check=True,
                            )

                    if j % 5 == 4 or j == 31:
                        nt = (j - base + 1) * 4      # t's in this window
                        t0 = base * 4
                        tmp = tpool.tile([128, 20, V], F32)
                        # tmp = po*scale + x   (residual; x strided v=32->25)
                        if _BISECT_NO_STT:
                            nc.vector.tensor_copy(
                                tmp[:, :nt, :],
                                po[:, 0:500].rearrange(
                                    "p (t w) -> p t w", w=V)[:, :nt, :],
                            )
                        else:
                            nc.vector.scalar_tensor_tensor(
                                tmp[:, :nt, :],
                                po[:, 0:500].rearrange(
                                    "p (t w) -> p t w", w=V)[:, :nt, :],
                                scale_sb,
                                x_bf[:, t0:t0 + nt, :V],
                                mybir.AluOpType.mult,
                                mybir.AluOpType.add,
                            )
                        # out = relu(tmp + shift)
                        nc.scalar.activation(
                            out_sb[:, t0:t0 + nt, :], tmp[:, :nt, :],
                            mybir.ActivationFunctionType.Relu,
                            bias=shift_sb,
                        )

                nc.sync.dma_start(
                    out=out_d[4 * q:4 * q + 4].rearrange(
                        "n c (t v) -> (n c) t v", v=V),
                    in_=out_sb,
                )

    return nc


def _prep_host(x, A, attn, Wd, bd, bn_gamma, bn_beta, bn_mean, bn_var):
    x = np.asarray(x, dtype=np.float32)
    A = np.asarray(A, dtype=np.float32)
    attn = np.asarray(attn, dtype=np.float32)
    Wd = np.asarray(Wd, dtype=np.float32)
    bd = np.asarray(bd, dtype=np.float32)
    bn_gamma = np.asarray(bn_gamma, dtype=np.float32)
    bn_beta = np.asarray(bn_beta, dtype=np.float32)
    bn_mean = np.asarray(bn_mean, dtype=np.float32)
    bn_var = np.asarray(bn_var, dtype=np.float32)

    # pad x's v dim 25 -> 32
    xp = np.zeros((N, C, T, VP), dtype=np.float32)
    xp[:, :, :, :V] = x.reshape(N, C, T, V)
    xp = xp.reshape(N, C, TVP)

    scale = bn_gamma / np.sqrt(bn_var + BN_EPS)           # [32]
    shift = (bd.sum(axis=0) - bn_mean) * scale + bn_beta  # [32]

    # block-diagonal Ah: bdall[n, 32q+v, 100h+25q'+w] = (A*attn)[n,h,v,w] δqq'
    Ah = A[None, :, :, :] * attn                  # [N,H,V,V]
    z6 = np.zeros((N, 4, VP, H, 4, V), dtype=np.float32)
    for qq in range(4):
        z6[:, qq, :V, :, qq, :] = Ah.transpose(0, 2, 1, 3)   # [n, v, h, w]
    bdall = z6.reshape(N, 128, 300).astype(ml_bf16())

    # wd4: block-diag [ (i c), (4i x 3h x 32o) ]: wd4[32i+c, 96i+32h+o] = Wd[h,o,c]
    wdt = Wd.transpose(2, 0, 1).reshape(C, H * C)         # [c, (h o)]
    wd4 = np.zeros((128, 384), dtype=np.float32)
    for i in range(4):
        wd4[32 * i:32 * (i + 1), 96 * i:96 * (i + 1)] = wdt
    wd4 = wd4.astype(ml_bf16())                           # [128, 384]

    scale4 = np.tile(scale, 4).reshape(128, 1).astype(np.float32)
    shift4 = np.tile(shift, 4).reshape(128, 1).astype(np.float32)
    return xp, bdall, wd4, scale4, shift4


def ml_bf16():
    import ml_dtypes
    return ml_dtypes.bfloat16


def kernel(x, A, attn, Wd, bd, bn_gamma, bn_beta, bn_mean, bn_var,
           _trace=False):
    xp, bdall, wd4, scale4, shift4 = _prep_host(
        x, A, attn, Wd, bd, bn_gamma, bn_beta, bn_mean, bn_var)

    if "nc" not in _CACHE:
        _CACHE["nc"] = _build_graph()
    nc = _CACHE["nc"]

    in_maps = []
    for i in range(NCORES):
        sl = slice(i * NSH, (i + 1) * NSH)
        in_maps.append({
            "xp": xp[sl],
            "bdall": bdall[sl],
            "wd4": wd4,
            "scale": scale4,
            "shift": shift4,
        })

    res = run_bass_kernel_spmd(
        nc, in_maps, core_ids=list(range(NCORES)), trace=_trace,
    )
    out = np.concatenate([r["out"] for r in res.results], axis=0)
    out = out.reshape(N, C, T, V)
    if _trace:
        return out, res
    return out


# revision 15
# speedup vs baseline: 1.8103x; 1.0428x over previous
"""Trainium2 Bass kernel for nn_AFFWithCustomGCN (SA-GC block + BN + residual relu).

Math (per batch n):
    Ah[h]   = A[h] * attn[n,h]                         # [H,V,V]
    feat    = einsum('ctv,hvw->hctw', x[n], Ah)        # aggregate over v
    pre     = einsum('hctw,hoc->otw', feat, Wd) + bd.sum(0)
    out     = relu(bn(pre) + x[n])                     # relu(relu(y)) == relu(y)

Data-parallel over batch N=256 across 8 cores (32/core), processed in quads
(4 batches) to fill 128 partitions.  x stays unpadded; chunks cover 5 whole
t's (125 (t,v) columns) so no v-padding waste: 25 full chunks + one 3-t tail
chunk (75 cols) per quad.

Per quad on device (phase-split so the PE stream is dense and HAM stays warm):
  Phase 1 (26 chunks j):
      pt[(t5 v)<=125, 96i+32h+o] = x_bf[chunk].T @ wd4
    (wd4 block-diag over batches zeroes cross-batch terms; x chunk is the
    stationary operand), then one PSUM->SBUF bf16 copy per chunk
    (DVE/ACT by parity) -> zt.
  Phase 2 (26 chunks): per head h=0..2, 4 batches in PE col groups:
      po[(i o), (t5 w)] += zt[:, 96i+32h:+32].T @ BD_h
    accumulated over heads in PSUM; accumulation group opened once per
    4-chunk bank window (start on first chunk, stop on last).
  Epilogue per window: DVE: tmp = po*scale + x (residual add),
  ACT: out = relu(tmp + shift).  BN params and summed conv bias folded
  into scale/shift.
"""

import numpy as np

import concourse.bass as bass
import concourse.tile as tile
from concourse import mybir
from concourse import bass2jax as _b2j
from concourse.bass_utils import run_bass_kernel_spmd


def _split_multi_waits(bir_json: bytes) -> bytes:
    """Walrus allows only one sync-wait per TPB instruction on several
    queue structs.  Split any instruction with >1 wait into preceding
    single-wait EventSemaphore instructions on the same engine (pure wait
    carriers, identical semantics)."""
    import orjson
    bir = orjson.loads(bir_json)
    ctr = 0
    for fn in bir.get("functions", []):
        for blk in fn.get("blocks", []):
            insts = blk.get("instructions")
            if not insts:
                continue
            out = []
            for inst in insts:
                si = inst.get("sync_info") or {}
                waits = si.get("on_wait") or []
                if len(waits) > 1:
                    eng = inst.get("engine")
                    for w in waits[:-1]:
                        out.append({
                            "debug": inst.get("debug", 0),
                            "engine": eng, "ins": [], "outs": [],
                            "name": f"WS-{ctr}",
                            "opcode": "EventSemaphore",
                            "sync_info": {"on_update": [], "on_wait": [w]},
                        })
                        ctr += 1
                    si["on_wait"] = [waits[-1]]
                out.append(inst)
            blk["instructions"] = out
    return orjson.dumps(bir)


_orig_compile_bir = _b2j.compile_bir_kernel


def _patched_compile_bir(bir_json, tmpdir, neff_name="file.neff"):
    return _orig_compile_bir(_split_multi_waits(bir_json), tmpdir,
                             neff_name=neff_name)


if _b2j.compile_bir_kernel is not _patched_compile_bir:
    _b2j.compile_bir_kernel = _patched_compile_bir

F32 = mybir.dt.float32
BF16 = mybir.dt.bfloat16

N, C, T, V, H = 256, 32, 128, 25, 3
TW = T * V                  # 3200
NCORES = 8
NSH = N // NCORES           # 32 batches per core
NQ = NSH // 4               # 8 quads per core
BN_EPS = 1e-5

# chunking: 5 whole t's per chunk (125 cols), 25 full + one 3-t tail
CHUNKS = [(125 * j, 125, 5 * j, 5) for j in range(25)] + [(3125, 75, 125, 3)]
# windows of 4 chunks sharing one PSUM bank (<=512 cols)
WINDOWS = [list(range(4 * w, 4 * w + 4)) for w in range(6)] + [[24, 25]]

_CACHE = {}


def _build_graph(nq=NQ):
    nc = bass.Bass()

    xp_d = nc.declare_dram_parameter("xp", [NSH, C, TW], F32, isOutput=False)
    bdall_d = nc.declare_dram_parameter("bdall", [NSH, 125, 375], BF16, isOutput=False)
    wd4_d = nc.declare_dram_parameter("wd4", [128, 384], BF16, isOutput=False)
    scale_d = nc.declare_dram_parameter("scale", [128, 1], F32, isOutput=False)
    shift_d = nc.declare_dram_parameter("shift", [128, 1], F32, isOutput=False)
    out_d = nc.declare_dram_parameter("out", [NSH, C, TW], F32, isOutput=True)

    with tile.TileContext(nc) as tc:
        with (
            tc.tile_pool(name="singles", bufs=1) as singles,
            tc.tile_pool(name="xpool", bufs=3) as xpool,
            tc.tile_pool(name="bdpool", bufs=2) as bdpool,
            tc.tile_pool(name="ztpool", bufs=48) as ztpool,
            tc.tile_pool(name="opool", bufs=2) as opool,
            tc.tile_pool(name="tpool", bufs=3) as tpool,
            tc.tile_pool(name="psPT", bufs=4, space="PSUM") as psPT,
            tc.tile_pool(name="psAcc", bufs=3, space="PSUM") as psAcc,
        ):
            wd4_sb = singles.tile([128, 384], BF16)
            nc.sync.dma_start(out=wd4_sb, in_=wd4_d[:, :])
            scale_sb = singles.tile([128, 1], F32)
            nc.sync.dma_start(out=scale_sb, in_=scale_d[:, :])
            shift_sb = singles.tile([128, 1], F32)
            nc.sync.dma_start(out=shift_sb, in_=shift_d[:, :])

            for q in range(nq):
                x_bf = xpool.tile([128, T, V], BF16)
                nc.gpsimd.dma_start(
                    out=x_bf,
                    in_=xp_d[4 * q:4 * q + 4].rearrange(
                        "n c (t v) -> (n c) t v", v=V),
                )
                # per-quad slice of the block-diag adjacency (tiny, loads
                # fast so phase 2 isn't gated on one big up-front DMA)
                bd_q = bdpool.tile([125, 4, 375], BF16)
                nc.sync.dma_start(
                    out=bd_q,
                    in_=bdall_d[4 * q:4 * q + 4].rearrange("n p f -> p n f"),
                )

                x_flat = x_bf.rearrange("p t v -> p (t v)")

                # Phase 1: channel-mix (x chunk stationary, wd4 streams)
                zts = []
                for (c0, cw, _, _) in CHUNKS:
                    pt = psPT.tile([128, 384], F32)
                    nc.tensor.matmul(
                        pt[0:cw, :],
                        x_flat[:, c0:c0 + cw],
                        wd4_sb,
                        start=True, stop=True,
                    )
                    zt = ztpool.tile([128, 384], BF16)
                    if len(zts) % 2 == 0:
                        nc.vector.tensor_copy(zt[0:cw, :], pt[0:cw, :])
                    else:
                        nc.scalar.activation(
                            zt[0:cw, :], pt[0:cw, :],
                            mybir.ActivationFunctionType.Copy)
                    zts.append(zt)

                # Phase 2: adjacency propagation + head-sum
                out_sb = opool.tile([128, T, V], F32)
                for w, chunk_ids in enumerate(WINDOWS):
                    po = psAcc.tile([128, 512], F32)
                    off = 0
                    t0 = CHUNKS[chunk_ids[0]][2]
                    for jj, j in enumerate(chunk_ids):
                        (c0, cw, tj, tn) = CHUNKS[j]
                        zt = zts[j]
                        first = (jj == 0)
                        last = (jj == len(chunk_ids) - 1)
                        for h in range(3):
                            for i in range(4):
                                nc.tensor.matmul(
                                    po[32 * i:32 * (i + 1), off:off + cw],
                                    zt[0:cw, 96 * i + 32 * h:96 * i + 32 * h + 32],
                                    bd_q[0:cw, i, 125 * h:125 * h + cw],
                                    start=(h == 0 and first),
                                    stop=(h == 2 and last),
                                    tile_position=(0, 32 * i),
                                    # CoreSim's group-check mis-addresses
                                    # base-partition!=0 col-tiled outs; the
                                    # pattern is HW-validated.
                                    skip_group_check=True,
                                )
                        off += cw

                    nt = off // V
                    tmp = tpool.tile([128, 20, V], F32)
                    # tmp = po*scale + x  (residual add)
                    nc.vector.scalar_tensor_tensor(
                        tmp[:, :nt, :],
                        po[:, 0:500].rearrange(
                            "p (t w) -> p t w", w=V)[:, :nt, :],
                        scale_sb,
                        x_bf[:, t0:t0 + nt, :],
                        mybir.AluOpType.mult,
                        mybir.AluOpType.add,
                    )
                    # out = relu(tmp + shift)
                    nc.scalar.activation(
                        out_sb[:, t0:t0 + nt, :], tmp[:, :nt, :],
                        mybir.ActivationFunctionType.Relu,
                        bias=shift_sb,
                    )

                # output DMA in halves so the first half streams out while
                # the second half is still being computed
                for (ta, tb) in ((0, 64), (64, 128)):
                    nc.sync.dma_start(
                        out=out_d[4 * q:4 * q + 4].rearrange(
                            "n c (t v) -> (n c) t v", v=V)[:, ta:tb, :],
                        in_=out_sb[:, ta:tb, :],
                    )

    return nc


def _prep_host(x, A, attn, Wd, bd, bn_gamma, bn_beta, bn_mean, bn_var):
    x = np.asarray(x, dtype=np.float32)
    A = np.asarray(A, dtype=np.float32)
    attn = np.asarray(attn, dtype=np.float32)
    Wd = np.asarray(Wd, dtype=np.float32)
    bd = np.asarray(bd, dtype=np.float32)
    bn_gamma = np.asarray(bn_gamma, dtype=np.float32)
    bn_beta = np.asarray(bn_beta, dtype=np.float32)
    bn_mean = np.asarray(bn_mean, dtype=np.float32)
    bn_var = np.asarray(bn_var, dtype=np.float32)

    xp = x.reshape(N, C, TW)

    scale = bn_gamma / np.sqrt(bn_var + BN_EPS)           # [32]
    shift = (bd.sum(axis=0) - bn_mean) * scale + bn_beta  # [32]

    # block-diagonal Ah over 5 t's: bdall[n, 25t+v, 125h+25t'+w] = Ah δtt'
    Ah = A[None, :, :, :] * attn                  # [N,H,V,V]
    z6 = np.zeros((N, 5, V, H, 5, V), dtype=np.float32)
    for tt in range(5):
        z6[:, tt, :, :, tt, :] = Ah.transpose(0, 2, 1, 3)   # [n, v, h, w]
    bdall = z6.reshape(N, 125, 375).astype(ml_bf16())

    # wd4: block-diag [ (i c), (4i x 3h x 32o) ]: wd4[32i+c, 96i+32h+o] = Wd[h,o,c]
    wdt = Wd.transpose(2, 0, 1).reshape(C, H * C)         # [c, (h o)]
    wd4 = np.zeros((128, 384), dtype=np.float32)
    for i in range(4):
        wd4[32 * i:32 * (i + 1), 96 * i:96 * (i + 1)] = wdt
    wd4 = wd4.astype(ml_bf16())                           # [128, 384]

    scale4 = np.tile(scale, 4).reshape(128, 1).astype(np.float32)
    shift4 = np.tile(shift, 4).reshape(128, 1).astype(np.float32)
    return xp, bdall, wd4, scale4, shift4


def ml_bf16():
    import ml_dtypes
    return ml_dtypes.bfloat16


def kernel(x, A, attn, Wd, bd, bn_gamma, bn_beta, bn_mean, bn_var,
           _trace=False):
    xp, bdall, wd4, scale4, shift4 = _prep_host(
        x, A, attn, Wd, bd, bn_gamma, bn_beta, bn_mean, bn_var)

    if "nc" not in _CACHE:
        _CACHE["nc"] = _build_graph()
    nc = _CACHE["nc"]

    in_maps = []
    for i in range(NCORES):
        sl = slice(i * NSH, (i + 1) * NSH)
        in_maps.append({
            "xp": xp[sl],
            "bdall": bdall[sl],
            "wd4": wd4,
            "scale": scale4,
            "shift": shift4,
        })

    res = run_bass_kernel_spmd(
        nc, in_maps, core_ids=list(range(NCORES)), trace=_trace,
    )
    out = np.concatenate([r["out"] for r in res.results], axis=0)
    out = out.reshape(N, C, T, V)
    if _trace:
        return out, res
    return out
